# revision 18
# baseline (speedup 1.0000x reference)
"""Fused ConvBNReLU1D (kernel_size=1) + per-tensor po2 weight/bias fake-quant
+ QuantReLU(8-bit unsigned) output fake-quant, on 8 Trainium2 NeuronCores.

Strategy (v2)
-------------
- Host: quantize W/b (per-tensor po2 scales, depend only on W/b). Ship x and
  Wq as bf16 (Wq is int8-levels * po2 scale => exactly representable in bf16;
  x bf16 rounding is far inside the rel-err budget). Halves DMA-in traffic.
- Device (SPMD, data-parallel over batch B=32 -> 4 batches/core):
  Phase A: y = relu(Wq @ x + bq) with bf16 matmuls (1 cycle/row), y kept
           resident in SBUF as bf16 (64 KiB/partition); per-chunk running
           maxes tracked on the vector engine.
  - AllGather of the per-partition max vector (cheaper than AllReduce in the
    collective fabric: no reduce pass) + local max reduce.
  Phase B: out = round(y*inv)*s. Rounding via the fp16 magic constant
           1.5*2^10=1536: y*inv in [0,255], +1536 lands in [1536,1791] where
           fp16's ulp is exactly 1.0 => RNE to integer on the fp16 output
           cast. stage1 (y*inv+1536 -> fp16) split across Act/Pool/DVE;
           stage2 ((t-1536)*s -> bf16) on DVE.
           Output DMA'd as bf16, upcast to fp32 on host.
"""

import os
import sys
from contextlib import ExitStack

import numpy as np
import ml_dtypes

for _p in ("/opt/trn_rl_repo", os.path.expanduser("~/.axon_site/_ro/trn_rl_repo")):
    if os.path.isdir(_p) and _p not in sys.path:
        sys.path.insert(0, _p)

import concourse.bacc as bacc
import concourse.mybir as mybir
import concourse.tile as tile
from concourse.bass_utils import run_bass_kernel_spmd

P = 128
B, CIN, COUT, N = 32, 512, 512, 2048
NCORES = 8
BSH = B // NCORES          # batches per core
NT = 512                   # matmul free dim (= one PSUM bank of fp32)
KT = CIN // P              # 4 contraction tiles
MT = COUT // P             # 4 output-row tiles
NJ = N // NT               # 4 n-windows per batch
NCH = BSH * NJ             # 16 (batch, n-window) chunks per core
CH2 = MT * NT              # columns of y per chunk (2048)
MAGIC16 = 1536.0           # 1.5 * 2^10: fp16 RNE rounding for t in [0, 256)
QMAX_S = 127.0
QMAX_U = 255.0

# phase-B stage1 engine per chunk: Act(11) / Pool(3) / DVE(2), spread out
STAGE1_ENG = ["A", "A", "P", "A", "A", "D", "A", "A",
              "P", "A", "A", "D", "A", "P", "A", "A"]

_cache = {}
LAST_RESULT = None         # BassKernelResults of the most recent run (test.py)


def _build():
    f32 = mybir.dt.float32
    bf16 = mybir.dt.bfloat16
    fp16 = mybir.dt.float16
    Relu = mybir.ActivationFunctionType.Relu
    Copy = mybir.ActivationFunctionType.Copy
    X = mybir.AxisListType.X
    Alu = mybir.AluOpType

    nc = bacc.Bacc(
        "TRN2",
        target_bir_lowering=False,
        debug=False,
        enable_asserts=False,
        num_devices=NCORES,
    )
    xs = nc.dram_tensor("xs", [BSH, CIN, N], bf16, kind="ExternalInput")
    # weights pre-packed on host to the exact SBUF layout: [P, KT*MT*P]
    wpk = nc.dram_tensor("wpk", [P, KT * MT * P], bf16, kind="ExternalInput")
    bqv = nc.dram_tensor("bqv", [P, MT], f32, kind="ExternalInput")
    out = nc.dram_tensor("out", [BSH, COUT, N], bf16, kind="ExternalOutput")

    with tile.TileContext(nc) as tc, ExitStack() as ctx:
        const = ctx.enter_context(tc.tile_pool(name="const", bufs=1))
        xpool = ctx.enter_context(tc.tile_pool(name="xp", bufs=4))
        ypool = ctx.enter_context(tc.tile_pool(name="yp", bufs=1))
        pspool = ctx.enter_context(tc.tile_pool(name="ps", bufs=7, space="PSUM"))
        psb = ctx.enter_context(tc.tile_pool(name="psb", bufs=1, space="PSUM"))
        t1pool = ctx.enter_context(tc.tile_pool(name="t1", bufs=4))
        t2pool = ctx.enter_context(tc.tile_pool(name="t2", bufs=4))
        dram = ctx.enter_context(tc.tile_pool(name="dram", bufs=1, space="DRAM"))

        wq = const.tile([P, KT * MT * P], bf16)

        def load_x_chunk(c, interleave_w=False):
            bb, j = divmod(c, NJ)
            xt = xpool.tile([P, KT * NT], bf16)
            if c < 4:
                # startup: per-k DMAs, weight k-quadrant just before x k-slice
                for k in range(KT):
                    if interleave_w:
                        nc.sync.dma_start(
                            out=wq[:, k * MT * P:(k + 1) * MT * P],
                            in_=wpk[:, k * MT * P:(k + 1) * MT * P],
                        )
                    # chunks 1-3 go through the Pool SWDGE queue: its desc-gen
                    # runs in parallel with HWDGE's startup backlog
                    eng = nc.sync if interleave_w else nc.gpsimd
                    eng.dma_start(
                        out=xt[:, k * NT:(k + 1) * NT],
                        in_=xs[bb, k * P:(k + 1) * P, j * NT:(j + 1) * NT],
                    )
            else:
                # steady state: one DMA per chunk (HWDGE desc-gen is a flat
                # ~625ns per DMA instruction - fewer, bigger DMAs)
                nc.sync.dma_start(
                    out=xt[:, :].rearrange("p (k n) -> p k n", k=KT),
                    in_=xs[bb, :, j * NT:(j + 1) * NT].rearrange(
                        "(k p) n -> p k n", p=P
                    ),
                )
            return xt

        xtiles = {0: load_x_chunk(0, interleave_w=True)}
        bias = const.tile([P, MT], f32)
        nc.gpsimd.dma_start(out=bias[:], in_=bqv[:, :])

        ybig = ypool.tile([P, NCH * CH2], bf16)
        maxb = const.tile([P, NCH * MT], f32)
        ones = const.tile([1, P], f32)
        nc.vector.memset(ones[:], 1.0)

        # ---- Phase A: y = relu(Wq @ x + bq), track per-column-block maxes.
        # k-outer so chunk 0's first matmul only needs the k=0 slices.
        LAST = NCH - 1
        for c in range(NCH):
            xt = xtiles.pop(c) if c in xtiles else load_x_chunk(c)
            pss = [pspool.tile([P, NT], f32, name="ps") for m in range(MT)]
            order = ([(k, m) for k in range(KT) for m in range(MT)] if c == 0
                     else [(k, m) for m in range(MT) for k in range(KT)])
            for k, m in order:
                nc.tensor.matmul(
                    pss[m][:],
                    wq[:, (k * MT + m) * P:(k * MT + m + 1) * P],
                    xt[:, k * NT:(k + 1) * NT],
                    start=(k == 0),
                    stop=(k == KT - 1),
                )
            if c == LAST:
                # critical path: max straight off PSUM (pre-bias, pre-relu);
                # max_n relu(z+b) == relu(b + max_n z) since both are monotone.
                # Act's y writes then run in parallel, off the critical path.
                mraw = const.tile([P, MT], f32)
                for m in range(MT):
                    nc.vector.reduce_max(mraw[:, m:m + 1], pss[m][:], axis=X)
                nc.vector.tensor_tensor(
                    out=maxb[:, c * MT:(c + 1) * MT], in0=mraw[:], in1=bias[:],
                    op=Alu.add,
                )
                nc.vector.tensor_scalar(
                    out=maxb[:, c * MT:(c + 1) * MT],
                    in0=maxb[:, c * MT:(c + 1) * MT],
                    scalar1=0.0, scalar2=None, op0=Alu.max,
                )
            for m in range(MT):
                col = (c * MT + m) * NT
                nc.scalar.activation(
                    ybig[:, col:col + NT], pss[m][:], Relu, bias=bias[:, m:m + 1]
                )
                if c != LAST:
                    nc.vector.reduce_max(
                        maxb[:, c * MT + m:c * MT + m + 1],
                        ybig[:, col:col + NT],
                        axis=X,
                    )

        # ---- Global max across cores (scale is global): AllGather + local max
        mloc = const.tile([P, 1], f32)
        nc.vector.reduce_max(mloc[:], maxb[:], axis=X)
        cc_in = dram.tile([1, P], f32)
        cc_out = dram.tile([NCORES, P], f32)
        nc.sync.dma_start(out=cc_in[:].rearrange("a b -> b a"), in_=mloc[:])
        nc.gpsimd.collective_compute(
            "AllGather",
            Alu.bypass,
            replica_groups=[list(range(NCORES))],
            ins=[cc_in.opt()],
            outs=[cc_out.opt()],
        )
        grow = const.tile([1, NCORES * P], f32)
        nc.sync.dma_start(
            out=grow[:], in_=cc_out[:, :].rearrange("a b -> () (a b)")
        )

        # sc columns: 0=gmax, 3=1/gmax, 5=inv=255/gmax, 6=s=gmax/255
        sc = const.tile([1, 8], f32)
        nc.vector.reduce_max(sc[0:1, 0:1], grow[:], axis=X)
        nc.vector.reciprocal(sc[0:1, 3:4], sc[0:1, 0:1])
        nc.vector.tensor_scalar(
            out=sc[0:1, 5:6], in0=sc[0:1, 3:4],
            scalar1=QMAX_U, scalar2=None, op0=Alu.mult,
        )
        nc.vector.tensor_scalar(
            out=sc[0:1, 6:7], in0=sc[0:1, 0:1],
            scalar1=1.0 / QMAX_U, scalar2=None, op0=Alu.mult,
        )
        nc.vector.tensor_scalar(
            out=sc[0:1, 7:8], in0=sc[0:1, 6:7],
            scalar1=-MAGIC16, scalar2=None, op0=Alu.mult,
        )

        # broadcast [inv, s] to all 128 partitions via a K=1 matmul with ones;
        # phase B reads the scales directly from PSUM (no SBUF copy)
        psc = psb.tile([P, 3], f32)
        nc.tensor.matmul(psc[:], ones[:], sc[0:1, 5:8], start=True, stop=True)
        scal = const.tile([P, 3], f32)
        nc.vector.tensor_copy(scal[:], psc[:])

        # ---- Phase B: out = round(y*inv)*s via fp16 magic-constant RNE.
        # stage1: t = fp16(y*inv + 1536)  [RNE-to-integer via fp16 ulp=1]
        # stage2: out_bf16 = (t-1536)*s; on Act as Copy(t*s + (-1536*s)).
        # Per-chunk engine pairs balance Act(0.83ns/el) / DVE(1.04) /
        # Pool(1.04/0.6) so no engine exceeds the ~23us output-DMA window.
        S1 = ["A","A","P","A","A","D","A","A","P","A","A","D","A","P","A","A"]
        S2 = ["D","D","D","D","D","D","D","D","D","D","D","D","D","D","D","D"]
        for c in range(NCH):
            bb, j = divmod(c, NJ)
            yc = ybig[:, c * CH2:(c + 1) * CH2]
            t16 = t1pool.tile([P, CH2], fp16)
            if S1[c] == "A":
                nc.scalar.activation(
                    t16[:], yc, Copy, bias=MAGIC16, scale=scal[:, 0:1]
                )
            else:
                e = nc.gpsimd if S1[c] == "P" else nc.vector
                e.tensor_scalar(
                    out=t16[:], in0=yc,
                    scalar1=scal[:, 0:1], scalar2=MAGIC16,
                    op0=Alu.mult, op1=Alu.add,
                )
            tb = t2pool.tile([P, CH2], bf16)
            if S2[c] == "A":
                # (t-1536)*s >= 0, so Relu(t*s + (-1536*s)) is exact and
                # Relu (unlike Copy) accepts a per-partition bias AP
                nc.scalar.activation(
                    tb[:], t16[:], Relu, bias=scal[:, 2:3], scale=scal[:, 1:2]
                )
            else:
                e = nc.gpsimd if S2[c] == "P" else nc.vector
                e.tensor_scalar(
                    out=tb[:], in0=t16[:],
                    scalar1=-MAGIC16, scalar2=scal[:, 1:2],
                    op0=Alu.add, op1=Alu.mult,
                )
            nc.sync.dma_start(
                out=out[bb, :, j * NT:(j + 1) * NT].rearrange(
                    "(m p) n -> p m n", p=P
                ),
                in_=tb[:, :].rearrange("p (m n) -> p m n", m=MT),
            )
    nc.compile()  # bacc lowering: register allocation, DCE, nop-fusion
    return nc


def _quant_po2(v, qmax):
    # mirrors reference.fake_quant_signed_po2 in float32
    v = np.asarray(v, np.float32)
    qmax = np.float32(qmax)
    maxabs = np.max(np.abs(v)).astype(np.float32)
    ratio = np.float32(maxabs / qmax)
    s = np.exp2(np.ceil(np.log2(ratio))).astype(np.float32)
    return (np.round(np.clip(v / s, -qmax, qmax)).astype(np.float32) * s).astype(
        np.float32
    )


def kernel(x, W, b):
    global LAST_RESULT
    x = np.asarray(x, np.float32)
    W = np.asarray(W, np.float32)
    b = np.asarray(b, np.float32)
    assert x.shape == (B, CIN, N) and W.shape == (COUT, CIN) and b.shape == (COUT,)

    Wq = _quant_po2(W, QMAX_S)
    bq = _quant_po2(b, QMAX_S)
    # lhsT tile (k, m) = Wq.T[k*128:(k+1)*128, m*128:(m+1)*128], packed at
    # column (k*MT+m)*P -> contiguous [P, KT*MT*P] so the DMA is 4 KiB runs
    wT = Wq.T.reshape(KT, P, MT, P)
    wpk_h = np.ascontiguousarray(
        wT.transpose(1, 0, 2, 3).reshape(P, KT * MT * P)
    ).astype(ml_dtypes.bfloat16)
    bq_h = np.ascontiguousarray(bq.reshape(MT, P).T)      # [P, MT]
    xb = np.ascontiguousarray(x).astype(ml_dtypes.bfloat16)

    if "nc" not in _cache:
        _cache["nc"] = _build()
    nc = _cache["nc"]

    in_maps = [
        {"xs": xb[c * BSH:(c + 1) * BSH], "wpk": wpk_h, "bqv": bq_h}
        for c in range(NCORES)
    ]
    res = run_bass_kernel_spmd(nc, in_maps, core_ids=list(range(NCORES)))
    LAST_RESULT = res
    return np.concatenate(
        [np.asarray(res.results[c]["out"]) for c in range(NCORES)], axis=0
    ).astype(np.float32)


if __name__ == "__main__":
    rng = np.random.default_rng(0)
    x = rng.standard_normal((B, CIN, N), np.float32)
    W = (rng.standard_normal((COUT, CIN)) * 0.05).astype(np.float32)
    b = (rng.standard_normal((COUT,)) * 0.1).astype(np.float32)
    y = kernel(x=x, W=W, b=b)
    print("out", y.shape, y.dtype, float(y.min()), float(y.max()))
